# revision 1
# baseline (speedup 1.0000x reference)
"""Trainium2 Bass kernel for the IMU preintegration module.

Full inputs in, full outputs out; internally data-parallel over 8 NeuronCores
(512 batch rows per core).

Math: the scan's per-row state composes associatively as (R, b, d) with
    b = sum_t R_{1..t} a_t,   d = sum_t (S-1-t) R_{1..t} a_t   (raw units;
physical factors of dt are folded into scalars).  Composition of two units
A,B of n steps each:
    R = RA RB,  b = bA + RA bB,  d = dA + n bA + RA dB.
Per-step rotations are tiny (|w| dt ~ 0.01 rad), so:
  L0: groups of n0=4 steps reduce to (theta, b, rho) with first-order
      rotation:  theta = sum w (+ BCH pair term), b = V + (dt/4)(theta x u
      + p x aw)  [u = ramp-weighted a-sum, p = Haar mode of w, aw its
      weight], rho = 3a0+2a1+a2.
  SA: small-angle composition up to 64-step units:
      theta' = tA+tB + (dt/2) tA x tB,  b' = bA+bB + dt (tA x bB),
      rho' = rA + n bA + rB.
  QT: convert theta -> quaternion (2-3 term polys in h = |theta dt/2|^2),
      exact quaternion binary tree for the remaining 5 levels.
Validated in numpy against the jax reference: rel err ~3.7e-3 (gate 2e-2).

Perf notes (hardware-measured): DVE fp32 ops run 1 elem/cycle at read
stride <= 8B, ~1.5x slower at 16-48B, ~2x at 96B.  So ScalarE (otherwise
idle) deinterleaves the accel channels into dense tiles; DVE reads only
dense / stride-2 data except the two w pair-sum ops.  GpSimd takes the
independent V-chain + one cross per slab.  SBUF: 8 rotating 6KB plane
buffers (Q1-Q8) hold all group-level 3-vectors via explicit reuse.
"""

import math
import os
import numpy as np

import concourse.mybir as mybir
from concourse import bass, bacc
from concourse.tile import TileContext

F32 = mybir.dt.float32
BF16 = mybir.dt.bfloat16
OP = mybir.AluOpType
AF = mybir.ActivationFunctionType

# problem constants (hardcoded per harness contract)
B_FULL = 4096
S_FULL = 2048
C = 6
N_CORES = 8
R = B_FULL // N_CORES          # rows per core = 512
DT = float(np.float32(1.0 / 200.0))

QSGN = [(1, -1, -1, -1), (1, 1, 1, -1), (1, -1, 1, 1), (1, 1, -1, 1)]
QIDX = [(0, 1, 2, 3), (1, 0, 3, 2), (2, 3, 0, 1), (3, 2, 1, 0)]


def build_nc(rows=R, s_len=S_FULL, s_chunk=512):
    nc = bacc.Bacc(None, target_bir_lowering=False, debug=False)
    g = rows // 128                    # 4 rows per partition
    n_slabs = s_len // s_chunk         # 4
    G = s_chunk // 4                   # 128 4-step groups per slab
    U8 = s_chunk // 8                  # 64 8-step units per slab
    U16 = s_chunk // 16                # 32 16-step units per slab
    NU16 = s_len // 16                 # 128 16-step units per row
    x = nc.dram_tensor("x", [rows, s_len, C], F32, kind="ExternalInput")
    out = nc.dram_tensor("out", [rows, 7], F32, kind="ExternalOutput")
    xv = x.rearrange("(g p) s c -> g p s c", g=g)

    V = nc.vector
    Gp = nc.gpsimd
    Sc = nc.scalar

    def comps(t, n=3):
        return [t[:, :, ci, :] for ci in range(n)]

    def cross_into(eng, dst, u, v, tmps):
        """dst = u x v per-component (lists of [128,g,U] views). 9 ops."""
        for i in range(3):
            i1, i2 = (i + 1) % 3, (i + 2) % 3
            eng.tensor_tensor(out=tmps[0], in0=u[i1], in1=v[i2], op=OP.mult)
            eng.tensor_tensor(out=tmps[1], in0=u[i2], in1=v[i1], op=OP.mult)
            eng.tensor_tensor(out=dst[i], in0=tmps[0], in1=tmps[1],
                              op=OP.subtract)

    def qmul_into(nq, q1, q2, tmps):
        """nq = q1 (x) q2 elementwise (lists of 4 views). 28 DVE ops."""
        for comp in range(4):
            acc = nq[comp]
            V.tensor_tensor(out=acc, in0=q1[0], in1=q2[QIDX[comp][0]],
                            op=OP.mult)
            for t in range(1, 4):
                tmp = tmps[t % 2]
                V.tensor_tensor(out=tmp, in0=q1[t], in1=q2[QIDX[comp][t]],
                                op=OP.mult)
                V.tensor_tensor(out=acc, in0=acc, in1=tmp,
                                op=OP.add if QSGN[comp][t] > 0 else OP.subtract)

    with TileContext(nc) as tc:
        with (
            tc.tile_pool(name="slab", bufs=2) as slab_pool,
            tc.tile_pool(name="deint", bufs=1) as deint_pool,
            tc.tile_pool(name="plane", bufs=1) as plane_pool,
            tc.tile_pool(name="xtmp", bufs=2) as xtmp_pool,
            tc.tile_pool(name="acc", bufs=1) as acc_pool,
            tc.tile_pool(name="tree", bufs=1) as tree_pool,
        ):
            pshape = [128, g, 3, G]

            def plane(buf, nm):
                return plane_pool.tile(pshape, F32, tag=f"q{buf}", name=nm)

            def vtmp(n=None):
                return [xtmp_pool.tile([128, g, G], F32, tag="vt",
                                       name="vt")[:, :, 0:(n or G)]
                        for _ in range(2)]

            def gtmp(n=None):
                return [xtmp_pool.tile([128, g, G], F32, tag="gt",
                                       name="gt")[:, :, 0:(n or G)]
                        for _ in range(2)]

            # persistent accumulators: 16-step units across all slabs
            th16 = acc_pool.tile([128, g, 3, NU16], F32, tag="t16",
                                 name="t16")
            b16 = acc_pool.tile([128, g, 3, NU16], F32, tag="bb16",
                                name="bb16")
            r16 = acc_pool.tile([128, g, 3, NU16], F32, tag="r16", name="r16")

            def sa_fold(oth, cs, scal):
                # oth += scal * cs, per component (stt needs <=2 free dims)
                for ci in range(3):
                    V.scalar_tensor_tensor(out=oth[:, :, ci, :],
                                           in0=cs[:, :, ci, :], scalar=scal,
                                           in1=oth[:, :, ci, :],
                                           op0=OP.mult, op1=OP.add)

            def sa_level(thI, bI, rI, n_steps, oth, ob, orr, cs, cs2, tmp,
                         radd_eng):
                thAc = [thI[:, :, ci, 0::2] for ci in range(3)]
                thBc = [thI[:, :, ci, 1::2] for ci in range(3)]
                bBc = [bI[:, :, ci, 1::2] for ci in range(3)]
                cross_into(V, comps(cs), thAc, thBc, tmp)
                V.tensor_tensor(out=oth, in0=thI[:, :, :, 0::2],
                                in1=thI[:, :, :, 1::2], op=OP.add)
                sa_fold(oth, cs, DT / 2.0)
                cross_into(V, comps(cs2), thAc, bBc, tmp)
                V.tensor_tensor(out=ob, in0=bI[:, :, :, 0::2],
                                in1=bI[:, :, :, 1::2], op=OP.add)
                sa_fold(ob, cs2, DT)
                radd_eng.tensor_tensor(out=orr, in0=rI[:, :, :, 0::2],
                                       in1=rI[:, :, :, 1::2], op=OP.add)
                for ci in range(3):
                    V.scalar_tensor_tensor(out=orr[:, :, ci, :],
                                           in0=bI[:, :, ci, 0::2],
                                           scalar=float(n_steps),
                                           in1=orr[:, :, ci, :],
                                           op0=OP.mult, op1=OP.add)

            for si in range(n_slabs):
                slab = slab_pool.tile([128, g, s_chunk, C], F32, tag="slab",
                                      name=f"slab{si}")
                for gi in range(g):
                    nc.sync.dma_start(
                        out=slab[:, gi],
                        in_=xv[gi, :, si * s_chunk:(si + 1) * s_chunk, :],
                    )

                # ScalarE: deinterleave accel channels into dense tiles.
                # Order At1, At0 first: the DVE u-chain reads them first.
                At = [deint_pool.tile(pshape, F32, tag=f"at{i}",
                                      name=f"at{i}") for i in range(4)]
                for i in (1, 0, 2, 3):
                    for ci in range(3):
                        Sc.copy(At[i][:, :, ci, :], slab[:, :, i::4, 3 + ci])

                # DVE: w pair sums (strided slab reads), theta (dense),
                # p directly in bf16 (only feeds the bf16 correction cross)
                W = [slab[:, :, i::4, 0:3].transpose([0, 1, 3, 2])
                     for i in range(4)]
                s01 = plane(1, "s01")
                s23 = plane(2, "s23")
                th4 = plane(3, "th4")
                q4h = plane(4, "q4h")[:].bitcast(BF16)   # two bf16 planes
                p4h = q4h[:, :, :, 0:G]
                u4h = q4h[:, :, :, G:2 * G]
                q5h = plane(5, "q5h")[:].bitcast(BF16)
                th4h = q5h[:, :, :, 0:G]
                awh = q5h[:, :, :, G:2 * G]
                V.tensor_tensor(out=s01[:], in0=W[0], in1=W[1], op=OP.add)
                V.tensor_tensor(out=s23[:], in0=W[2], in1=W[3], op=OP.add)
                V.tensor_tensor(out=th4[:], in0=s01[:], in1=s23[:], op=OP.add)
                V.tensor_tensor(out=p4h, in0=s01[:], in1=s23[:],
                                op=OP.subtract)
                Sc.copy(th4h, th4[:])

                # DVE: BCH cross c3 = s01 x s23 in bf16 (cheap, no
                # cross-engine stall on the theta fold)
                c3h = plane_pool.tile([128, g, 3, G], BF16, tag="c3h",
                                      name="c3h")
                gt = [xtmp_pool.tile([128, g, G], F32, tag="gt",
                                     name="gt")[:].bitcast(BF16)[:, :, 0:G]
                      for _ in range(2)]
                cross_into(V, comps(c3h), comps(s01), comps(s23), gt)
                # GpSimd: V-chain over At (dense)
                vv = plane(8, "vv")
                Gp.tensor_tensor(out=vv[:], in0=At[0][:], in1=At[1][:],
                                 op=OP.add)
                Gp.tensor_tensor(out=vv[:], in0=vv[:], in1=At[2][:],
                                 op=OP.add)
                Gp.tensor_tensor(out=vv[:], in0=vv[:], in1=At[3][:],
                                 op=OP.add)

                # DVE: u-chain / aw / rho on dense At (flattened stt)
                t0 = plane(6, "t0")
                V.scalar_tensor_tensor(out=t0[:].opt(), in0=At[1][:].opt(),
                                       scalar=2.0, in1=At[0][:].opt(),
                                       op0=OP.mult, op1=OP.add)
                u4f = plane(7, "u4f")
                V.scalar_tensor_tensor(out=u4f[:].opt(), in0=At[2][:].opt(),
                                       scalar=3.0, in1=t0[:].opt(),
                                       op0=OP.mult, op1=OP.add)
                V.scalar_tensor_tensor(out=u4h.opt(), in0=At[3][:].opt(),
                                       scalar=4.0, in1=u4f[:].opt(),
                                       op0=OP.mult, op1=OP.add)
                # bf16 correction crosses (2x DVE mode)
                q2h = plane(2, "q2h")       # reuse s23 (dead after th/p)
                q2hb = q2h[:].bitcast(BF16)
                c1h = q2hb[:, :, :, 0:G]
                c2h = q2hb[:, :, :, G:2 * G]
                btmp = [xtmp_pool.tile([128, g, G], F32, tag="vt",
                                       name="vt")[:].bitcast(BF16)[:, :, 0:G]
                        for _ in range(2)]
                cross_into(V, comps(c1h), comps(th4h), comps(u4h), btmp)
                # aw = t0 + a2 (in place on t0)
                V.tensor_tensor(out=t0[:], in0=t0[:], in1=At[2][:], op=OP.add)
                aw = t0
                Sc.copy(awh, aw[:])
                # c2 = p4 x aw (bf16)
                cross_into(V, comps(c2h), comps(p4h), comps(awh), btmp)
                # rho = 2 a0 + aw  (u4f dead after its cast -> reuse Q7)
                rho4 = plane(7, "rho4")
                V.scalar_tensor_tensor(out=rho4[:].opt(), in0=At[0][:].opt(),
                                       scalar=2.0, in1=aw[:].opt(),
                                       op0=OP.mult, op1=OP.add)

                # folds: theta += (dt/2) c3 ; b = V + (dt/4)(c1+c2)
                V.scalar_tensor_tensor(out=th4[:].opt(), in0=c3h[:].opt(),
                                       scalar=DT / 2.0, in1=th4[:].opt(),
                                       op0=OP.mult, op1=OP.add)
                V.tensor_tensor(out=c1h, in0=c1h, in1=c2h, op=OP.add)
                V.scalar_tensor_tensor(out=vv[:].opt(), in0=c1h.opt(),
                                       scalar=DT / 4.0, in1=vv[:].opt(),
                                       op0=OP.mult, op1=OP.add)
                b4 = vv

                # ---- SA1: n=4 -> 8 ----
                th8 = plane(1, "th8")[:, :, :, 0:U8]   # s01 dead
                b8v = plane(2, "b8")[:, :, :, 0:U8]    # c1h/c2h dead
                r8v = plane(5, "r8")[:, :, :, 0:U8]    # th4h/awh dead
                csa = plane(4, "csa")[:, :, :, 0:U8]   # p4h/u4h dead
                csb = plane(6, "csb")[:, :, :, 0:U8]   # aw dead
                sa_level(th4[:], b4[:], rho4[:], 4, th8, b8v, r8v,
                         csa, csb, vtmp(U8), Gp)

                # ---- SA2: n=8 -> 16, into accumulators ----
                oth = th16[:, :, :, si * U16:(si + 1) * U16]
                ob = b16[:, :, :, si * U16:(si + 1) * U16]
                orr = r16[:, :, :, si * U16:(si + 1) * U16]
                csa2 = plane(4, "csa2")[:, :, :, 0:U16]
                csb2 = plane(6, "csb2")[:, :, :, 0:U16]
                sa_level(th8, b8v, r8v, 8, oth, ob, orr,
                         csa2, csb2, vtmp(U16), Gp)

            # ---- batched SA levels: 16->32->64 ----
            th32 = plane(1, "th32")[:, :, :, 0:64]
            b32 = plane(2, "b32")[:, :, :, 0:64]
            r32 = plane(5, "r32")[:, :, :, 0:64]
            cs_a = plane(4, "cs_a")[:, :, :, 0:64]
            cs_b = plane(6, "cs_b")[:, :, :, 0:64]
            sa_level(th16[:], b16[:], r16[:], 16, th32, b32, r32,
                     cs_a, cs_b, vtmp(64), Gp)
            th64 = plane(3, "th64")[:, :, :, 0:32]
            bd64 = plane_pool.tile([128, g, 2, 3, 32], F32, tag="q7",
                                     name="bd64")
            b64 = bd64[:, :, 0]
            r64 = bd64[:, :, 1]
            cs_a2 = plane(4, "cs_a2")[:, :, :, 0:32]
            cs_b2 = plane(6, "cs_b2")[:, :, :, 0:32]
            sa_level(th32, b32, r32, 32, th64, b64, r64,
                     cs_a2, cs_b2, vtmp(32), Gp)

            # ---- convert 64-step units to quaternions ----
            NU = 32
            K2 = (DT / 2.0) ** 2
            h2 = tree_pool.tile([128, g, NU], F32, tag="h2", name="h2")
            hy = tree_pool.tile([128, g, NU], F32, tag="hy", name="hy")
            hz = tree_pool.tile([128, g, NU], F32, tag="hz", name="hz")
            q64 = tree_pool.tile([128, g, 4, NU], F32, tag="q64", name="q64")
            V.tensor_tensor(out=h2[:], in0=th64[:, :, 0, :],
                            in1=th64[:, :, 0, :], op=OP.mult)
            for ci in (1, 2):
                V.tensor_tensor(out=hy[:], in0=th64[:, :, ci, :],
                                in1=th64[:, :, ci, :], op=OP.mult)
                V.tensor_tensor(out=h2[:], in0=h2[:], in1=hy[:], op=OP.add)
            # qw = 1 - (k/2) h2 + (k^2/24) h2^2
            V.scalar_tensor_tensor(out=hy[:], in0=h2[:],
                                   scalar=K2 * K2 / 24.0,
                                   in1=h2[:], op0=OP.mult, op1=OP.mult)
            V.scalar_tensor_tensor(out=hz[:], in0=h2[:], scalar=-K2 / 2.0,
                                   in1=hy[:], op0=OP.mult, op1=OP.add)
            Sc.activation(q64[:, :, 0, :], hz[:], AF.Identity, bias=1.0)
            # qv = (dt/2)(1 - (k/6) h2 + (k^2/120) h2^2) * theta
            V.scalar_tensor_tensor(out=hy[:], in0=h2[:],
                                   scalar=K2 * K2 / 120.0,
                                   in1=h2[:], op0=OP.mult, op1=OP.mult)
            V.scalar_tensor_tensor(out=hz[:], in0=h2[:], scalar=-K2 / 6.0,
                                   in1=hy[:], op0=OP.mult, op1=OP.add)
            Sc.activation(hz[:], hz[:], AF.Copy, scale=DT / 2.0,
                          bias=DT / 2.0)
            for ci in range(3):
                V.tensor_tensor(out=q64[:, :, 1 + ci, :],
                                in0=th64[:, :, ci, :], in1=hz[:], op=OP.mult)

            # ---- quaternion binary tree over 32 units (5 levels) ----
            # qmul via 16 ops (outer-product + signed gathers); b and d
            # rotated together as a stacked [2,3] block with stride-0
            # broadcast of the left quaternion.
            qt = q64
            bdt = bd64
            n2 = 64.0
            n_lvl = int(math.log2(NU))
            for lvl in range(1, n_lvl + 1):
                n = NU >> lvl
                nqt = tree_pool.tile([128, g, 4, n], F32, tag=f"nq{lvl}",
                                     name=f"nq{lvl}")
                nbd = tree_pool.tile([128, g, 2, 3, n], F32, tag=f"nbd{lvl}",
                                     name=f"nbd{lvl}")
                # temps live in dead plane buffers (all L0/SA planes are
                # free during the tree)
                P = [plane_pool.tile([128, g, 4, n], F32, tag=f"q{a + 1}",
                                     name=f"P{a}")[:] for a in range(4)]
                tcc = plane_pool.tile([128, g, 2, 3, n], F32, tag="q6",
                                      name="tcc")
                tcq = tree_pool.tile([128, g, 2, 3, n], F32, tag="tcq",
                                     name="tcq")
                tcw = plane_pool.tile([128, g, 2, 3, n], F32, tag="q8",
                                      name="tcw")
                tms = [xtmp_pool.tile([128, g, G], F32, tag="vt",
                                      name="tms")[:, :, 0:2 * n].rearrange(
                    "p g (s n) -> p g s n", s=2) for i in range(2)]

                q2all = qt[:, :, :, 1::2]
                for a in range(4):
                    q1a = qt[:, :, a, 0::2].unsqueeze(2).broadcast_to(
                        [128, g, 4, n])
                    V.tensor_tensor(out=P[a], in0=q1a, in1=q2all, op=OP.mult)
                for c in range(4):
                    idx = QIDX[c]
                    sgn = QSGN[c]
                    acc = nqt[:, :, c, :]
                    V.tensor_tensor(
                        out=acc, in0=P[0][:, :, idx[0], :],
                        in1=P[1][:, :, idx[1], :],
                        op=OP.add if sgn[1] > 0 else OP.subtract)
                    for t in (2, 3):
                        V.tensor_tensor(
                            out=acc, in0=acc, in1=P[t][:, :, idx[t], :],
                            op=OP.add if sgn[t] > 0 else OP.subtract)

                def qb(comp):
                    return qt[:, :, comp, 0::2].unsqueeze(2).broadcast_to(
                        [128, g, 2, n])

                def bd2(comp):
                    return bdt[:, :, :, comp, 1::2]

                # tcc = qv1 x bd2 ; tcq = q0 * bd2 ; inner = tcc + tcq
                for i in range(3):
                    i1, i2 = (i + 1) % 3, (i + 2) % 3
                    V.tensor_tensor(out=tms[0], in0=qb(1 + i1), in1=bd2(i2),
                                    op=OP.mult)
                    V.tensor_tensor(out=tms[1], in0=qb(1 + i2), in1=bd2(i1),
                                    op=OP.mult)
                    V.tensor_tensor(out=tcc[:, :, :, i, :], in0=tms[0],
                                    in1=tms[1], op=OP.subtract)
                for i in range(3):
                    V.tensor_tensor(out=tcq[:, :, :, i, :], in0=qb(0),
                                    in1=bd2(i), op=OP.mult)
                V.tensor_tensor(out=tcc[:].opt(), in0=tcc[:].opt(),
                                in1=tcq[:].opt(), op=OP.add)
                # tcw = qv1 x inner
                for i in range(3):
                    i1, i2 = (i + 1) % 3, (i + 2) % 3
                    V.tensor_tensor(out=tms[0], in0=qb(1 + i1),
                                    in1=tcc[:, :, :, i2, :], op=OP.mult)
                    V.tensor_tensor(out=tms[1], in0=qb(1 + i2),
                                    in1=tcc[:, :, :, i1, :], op=OP.mult)
                    V.tensor_tensor(out=tcw[:, :, :, i, :], in0=tms[0],
                                    in1=tms[1], op=OP.subtract)
                # nbd = bd1 + bd2 + 2*tcw  (+ n2*b1 on the d slot)
                V.tensor_tensor(out=nbd[:].opt(),
                                in0=bdt[:, :, :, :, 0::2].opt(),
                                in1=bdt[:, :, :, :, 1::2].opt(), op=OP.add)
                V.tensor_tensor(out=nbd[:].opt(), in0=nbd[:].opt(),
                                in1=tcw[:].opt(), op=OP.add)
                V.tensor_tensor(out=nbd[:].opt(), in0=nbd[:].opt(),
                                in1=tcw[:].opt(), op=OP.add)
                for c in range(3):
                    V.scalar_tensor_tensor(out=nbd[:, :, 1, c, :],
                                           in0=bdt[:, :, 0, c, 0::2],
                                           scalar=n2,
                                           in1=nbd[:, :, 1, c, :],
                                           op0=OP.mult, op1=OP.add)

                qt, bdt = nqt, nbd
                n2 *= 2.0

            cur_q = [qt[:, :, ci, :] for ci in range(4)]
            cur_b = [bdt[:, :, 0, ci, :] for ci in range(3)]
            cur_d = [bdt[:, :, 1, ci, :] for ci in range(3)]

            # ---- finalize ----
            out_t = tree_pool.tile([128, g, 7], F32, tag="outt",
                                   name="outt")[:]
            tf = tree_pool.tile([128, g, 1], F32, tag="tf", name="tf")[:]
            for i in range(3):
                V.scalar_tensor_tensor(out=tf, in0=cur_b[i], scalar=1.5,
                                       in1=cur_d[i], op0=OP.mult, op1=OP.add)
                V.tensor_scalar(out=out_t[:, :, i:i + 1], in0=tf,
                                scalar1=DT * DT, scalar2=None, op0=OP.mult)
            sg = tree_pool.tile([128, g, 1], F32, tag="sg", name="sg")[:]
            V.tensor_scalar(out=sg, in0=cur_q[0], scalar1=0.0, scalar2=2.0,
                            op0=OP.is_ge, op1=OP.mult)
            V.tensor_scalar(out=sg, in0=sg, scalar1=-1.0, scalar2=None,
                            op0=OP.add)
            for i in range(4):
                V.tensor_tensor(out=out_t[:, :, 3 + i:4 + i], in0=cur_q[i],
                                in1=sg, op=OP.mult)

            ov = out.rearrange("(g p) c -> g p c", g=g)
            for gi in range(g):
                nc.sync.dma_start(out=ov[gi], in_=out_t[:, gi, :])

    nc.compile()
    return nc


_NC_CACHE = {}
LAST_RESULTS = None


def _ensure_profiling_hooks():
    """Best-effort: provide the antenv.axon_hooks shim + skip S3 upload so
    trace=True works in this stripped container. No-op on failure."""
    import sys
    import types
    try:
        if "antenv.axon_hooks" not in sys.modules:
            from trn_agent_boot.trn_boot import _ntff_profile_via_ctypes
            hook = _ntff_profile_via_ctypes("/opt/axon/libaxon_pjrt.so")
            mod = types.ModuleType("antenv.axon_hooks")
            mod._hook = hook
            mod.get_axon_ntff_profile_hook = lambda: mod._hook
            mod.set_axon_ntff_profile_hook = lambda h: setattr(mod, "_hook", h)
            sys.modules["antenv.axon_hooks"] = mod
        import concourse.bass_utils as bu
        bu.upload_artifacts = lambda tmpdir: tmpdir
    except Exception as e:  # pragma: no cover
        print(f"profiling hook setup failed ({e}); tracing may be skipped")


def kernel(input_seq: np.ndarray) -> np.ndarray:
    from concourse.bass_utils import run_bass_kernel_spmd

    global LAST_RESULTS
    input_seq = np.ascontiguousarray(np.asarray(input_seq, dtype=np.float32))
    assert input_seq.shape == (B_FULL, S_FULL, C), input_seq.shape

    if "nc" not in _NC_CACHE:
        _NC_CACHE["nc"] = build_nc()
    nc = _NC_CACHE["nc"]

    in_maps = [{"x": input_seq[i * R:(i + 1) * R]} for i in range(N_CORES)]
    trace = os.environ.get("BASS_KERNEL_TRACE", "0") == "1"
    if trace:
        _ensure_profiling_hooks()
    try:
        res = run_bass_kernel_spmd(nc, in_maps, core_ids=list(range(N_CORES)),
                                   trace=trace)
    except Exception:
        # transient device wedge (NRT_EXEC_UNIT_UNRECOVERABLE) recovers on
        # a clean re-run; retry once
        res = run_bass_kernel_spmd(nc, in_maps, core_ids=list(range(N_CORES)),
                                   trace=trace)
    LAST_RESULTS = res
    return np.concatenate([r["out"] for r in res.results], axis=0)



# revision 6
# speedup vs baseline: 1.0522x; 1.0522x over previous
"""Trainium2 Bass kernel for the IMU preintegration module.

Full inputs in, full outputs out; internally data-parallel over 8 NeuronCores
(512 batch rows per core).

Math: the scan's per-row state composes associatively as (R, b, d) with
    b = sum_t R_{1..t} a_t,   d = sum_t (S-1-t) R_{1..t} a_t   (raw units;
physical factors of dt are folded into scalars).  Composition of two units
A,B of n steps each:
    R = RA RB,  b = bA + RA bB,  d = dA + n bA + RA dB.
Per-step rotations are tiny (|w| dt ~ 0.01 rad), so:
  L0: groups of n0=4 steps reduce to (theta, b, rho) with first-order
      rotation.  theta = s01+s23 (+ (dt/2) s01 x s23 BCH term) where
      s01/s23 are the step-pair sums of w.  The b correction
      (dt/4)(theta x u + p x aw) is algebraically identical to
      dt*(s01 x m1) + (dt/2)*(s23 x m2) with m1 = V - A0/2, m2 = A2+2*A3
      (V = 4-step a-sum, Ai = per-phase deinterleaved a) -- this form
      kills the ramp/haar chains of the straightforward derivation.
      rho = 3*A0 + 2*A1 + A2.
  SA: small-angle composition up to 256-step units:
      theta' = tA+tB + (dt/2) tA x tB,  b' = bA+bB + dt (tA x bB),
      rho' = rA + n bA + rB.
  QT: convert theta -> quaternion (2-3 term polys in h = |theta dt/2|^2),
      exact quaternion binary tree for the remaining 3 levels.

Perf notes (hardware-measured): DVE fp32 ops run 1 elem/cycle at read
stride <= 8B, ~2.3x slower at 96B.  ScalarE deinterleaves the accel
channels into dense bf16 tiles; GpSimd takes the V-chain, the m1/m2
correction vectors and all r-path adds.  Corrections and the 8/16-step
levels run in bf16 (2x DVE or half traffic).  Group-level tensors are
[128, 4, 3, N] tiles whose (g,c) dims are fused into a single free dim
for the adds/folds so scalar_tensor_tensor sees <= 2 free dims (1 op
instead of 3).
"""

import math
import os
import numpy as np

import concourse.mybir as mybir
from concourse import bass, bacc
from concourse.tile import TileContext

F32 = mybir.dt.float32
BF16 = mybir.dt.bfloat16
OP = mybir.AluOpType
AF = mybir.ActivationFunctionType

# problem constants (hardcoded per harness contract)
B_FULL = 4096
S_FULL = 2048
C = 6
N_CORES = 8
R = B_FULL // N_CORES          # rows per core = 512
DT = float(np.float32(1.0 / 200.0))

QSGN = [(1, -1, -1, -1), (1, 1, 1, -1), (1, -1, 1, 1), (1, 1, -1, 1)]
QIDX = [(0, 1, 2, 3), (1, 0, 3, 2), (2, 3, 0, 1), (3, 2, 1, 0)]

# step chunks per slab: short first chunk so compute starts early, short
# last chunk so the tree phase starts early
CHUNKS = [256, 512, 512, 512, 256]
assert sum(CHUNKS) == S_FULL


def fuse(ap):
    # [128, g, c, n] -> [128, g*c, n] so stt sees <= 2 free dims
    return ap.rearrange("p g c n -> p (g c) n")


def build_nc(rows=R, s_len=S_FULL):
    nc = bacc.Bacc(None, target_bir_lowering=False, debug=False)
    g = rows // 128                    # 4 rows per partition
    GMAX = 128                         # groups per slab at chunk=512
    NU16 = s_len // 16                 # 128 16-step units per row
    x = nc.dram_tensor("x", [rows, s_len, C], F32, kind="ExternalInput")
    out = nc.dram_tensor("out", [rows, 7], F32, kind="ExternalOutput")
    xv = x.rearrange("(g p) s c -> g p s c", g=g)

    V = nc.vector
    Gp = nc.gpsimd
    Sc = nc.scalar

    def cross_into(eng, dst, u, v, tmps):
        """dst = u x v per-component (lists of comp views). 9 ops."""
        for i in range(3):
            i1, i2 = (i + 1) % 3, (i + 2) % 3
            eng.tensor_tensor(out=tmps[0], in0=u[i1], in1=v[i2], op=OP.mult)
            eng.tensor_tensor(out=tmps[1], in0=u[i2], in1=v[i1], op=OP.mult)
            eng.tensor_tensor(out=dst[i], in0=tmps[0], in1=tmps[1],
                              op=OP.subtract)

    with TileContext(nc) as tc:
        with (
            tc.tile_pool(name="slab", bufs=2) as slab_pool,
            tc.tile_pool(name="deint", bufs=1) as deint_pool,
            tc.tile_pool(name="plane", bufs=1) as plane_pool,
            tc.tile_pool(name="half", bufs=1) as half_pool,
            tc.tile_pool(name="xtmp", bufs=2) as xtmp_pool,
            tc.tile_pool(name="acc", bufs=1) as acc_pool,
            tc.tile_pool(name="tree", bufs=1) as tree_pool,
        ):
            pshape = [128, g, 3, GMAX]

            def plane(buf, nm):
                return plane_pool.tile(pshape, F32, tag=f"q{buf}", name=nm)

            def hplane(buf, nm):
                return half_pool.tile(pshape, BF16, tag=f"h{buf}", name=nm)

            def btmp(n):
                return [xtmp_pool.tile([128, g, GMAX], F32, tag="vt",
                                       name="vt")[:].bitcast(BF16)[:, :, 0:n]
                        for _ in range(2)]

            # persistent accumulators: 16-step units across all slabs
            th16 = acc_pool.tile(pshape, F32, tag="t16", name="t16")
            b16 = acc_pool.tile(pshape, F32, tag="bb16", name="bb16")
            r16 = acc_pool.tile(pshape, F32, tag="r16", name="r16")

            def sa_level(thI, bI, rI, n_steps, oth, ob, orr, csa, csb, tmps):
                """Compose adjacent units (width 2U views -> width U).
                thI/bI/rI/oth/ob/orr/csa/csb are 4D [128,g,3,*] views;
                csa/csb bf16.  r-path runs on GpSimd."""
                thA = [thI[:, :, ci, 0::2] for ci in range(3)]
                thB = [thI[:, :, ci, 1::2] for ci in range(3)]
                bB = [bI[:, :, ci, 1::2] for ci in range(3)]
                cross_into(V, [csa[:, :, ci, :] for ci in range(3)],
                           thA, thB, tmps)
                V.tensor_tensor(out=fuse(oth), in0=fuse(thI)[:, :, 0::2],
                                in1=fuse(thI)[:, :, 1::2], op=OP.add)
                V.scalar_tensor_tensor(out=fuse(oth), in0=fuse(csa),
                                       scalar=DT / 2.0, in1=fuse(oth),
                                       op0=OP.mult, op1=OP.add)
                cross_into(V, [csb[:, :, ci, :] for ci in range(3)],
                           thA, bB, tmps)
                V.tensor_tensor(out=fuse(ob), in0=fuse(bI)[:, :, 0::2],
                                in1=fuse(bI)[:, :, 1::2], op=OP.add)
                V.scalar_tensor_tensor(out=fuse(ob), in0=fuse(csb),
                                       scalar=DT, in1=fuse(ob),
                                       op0=OP.mult, op1=OP.add)
                Gp.tensor_tensor(out=fuse(orr), in0=fuse(rI)[:, :, 0::2],
                                 in1=fuse(rI)[:, :, 1::2], op=OP.add)
                V.scalar_tensor_tensor(out=fuse(orr),
                                       in0=fuse(bI)[:, :, 0::2],
                                       scalar=float(n_steps), in1=fuse(orr),
                                       op0=OP.mult, op1=OP.add)

            s0 = 0
            for si, S_c in enumerate(CHUNKS):
                G = S_c // 4
                U8 = S_c // 8
                U16 = S_c // 16
                u16o = s0 // 16

                slab = slab_pool.tile([128, g, 512, C], F32, tag="slab",
                                      name=f"slab{si}")
                for gi in range(g):
                    nc.sync.dma_start(
                        out=slab[:, gi, 0:S_c, :],
                        in_=xv[gi, :, s0:s0 + S_c, :],
                    )

                # DVE: w pair sums (strided slab reads) + theta
                W = [slab[:, :, i:S_c:4, 0:3].transpose([0, 1, 3, 2])
                     for i in range(4)]
                s01 = plane(1, "s01")[:, :, :, 0:G]
                s23 = plane(2, "s23")[:, :, :, 0:G]
                th4 = plane(3, "th4")[:, :, :, 0:G]
                V.tensor_tensor(out=s01, in0=W[0], in1=W[1], op=OP.add)
                V.tensor_tensor(out=s23, in0=W[2], in1=W[3], op=OP.add)
                V.tensor_tensor(out=fuse(th4), in0=fuse(s01), in1=fuse(s23),
                                op=OP.add)

                # ScalarE: deinterleave accel channels into dense bf16
                # tiles, then cast s01/s23 for the pure-bf16 crosses
                At = [deint_pool.tile(pshape, BF16, tag=f"at{i}",
                                      name=f"at{i}")[:, :, :, 0:G]
                      for i in range(4)]
                for i in (0, 1, 2, 3):
                    for ci in range(3):
                        Sc.copy(At[i][:, :, ci, :],
                                slab[:, :, i:S_c:4, 3 + ci])
                s01h = hplane(1, "s01h")[:, :, :, 0:G]
                s23h = hplane(2, "s23h")[:, :, :, 0:G]
                Sc.copy(fuse(s01h), fuse(s01))
                Sc.copy(fuse(s23h), fuse(s23))

                # GpSimd: V-chain + correction vectors m1, m2 (bf16 out)
                vv = plane(4, "vv")[:, :, :, 0:G]
                Gp.tensor_tensor(out=fuse(vv), in0=fuse(At[0]),
                                 in1=fuse(At[1]), op=OP.add)
                Gp.tensor_tensor(out=fuse(vv), in0=fuse(vv),
                                 in1=fuse(At[2]), op=OP.add)
                Gp.tensor_tensor(out=fuse(vv), in0=fuse(vv),
                                 in1=fuse(At[3]), op=OP.add)
                # m2 = A2 + 2 A3 (GpSimd, pure bf16); m1 = 2V - A0 (DVE stt)
                m2h = hplane(3, "m2h")[:, :, :, 0:G]
                m1h = hplane(4, "m1h")[:, :, :, 0:G]
                Gp.tensor_tensor(out=fuse(m2h), in0=fuse(At[3]),
                                 in1=fuse(At[3]), op=OP.add)
                Gp.tensor_tensor(out=fuse(m2h), in0=fuse(m2h),
                                 in1=fuse(At[2]), op=OP.add)
                V.scalar_tensor_tensor(out=fuse(m1h), in0=fuse(vv),
                                       scalar=2.0, in1=fuse(At[0]),
                                       op0=OP.mult, op1=OP.subtract)

                # DVE: rho = 3 A0 + 2 A1 + A2 (two dense stt)
                r1 = plane(5, "r1")[:, :, :, 0:G]
                rho4 = plane(6, "rho4")[:, :, :, 0:G]
                V.scalar_tensor_tensor(out=fuse(r1), in0=fuse(At[0]),
                                       scalar=3.0, in1=fuse(At[2]),
                                       op0=OP.mult, op1=OP.add)
                V.scalar_tensor_tensor(out=fuse(rho4), in0=fuse(At[1]),
                                       scalar=2.0, in1=fuse(r1),
                                       op0=OP.mult, op1=OP.add)

                # DVE: BCH cross c3 = s01 x s23 (pure bf16) and theta fold
                c3h = hplane(5, "c3h")[:, :, :, 0:G]
                V_tmp = btmp(G)
                cross_into(V, [c3h[:, :, ci, :] for ci in range(3)],
                           [s01h[:, :, ci, :] for ci in range(3)],
                           [s23h[:, :, ci, :] for ci in range(3)], V_tmp)
                V.scalar_tensor_tensor(out=fuse(th4), in0=fuse(c3h),
                                       scalar=DT / 2.0, in1=fuse(th4),
                                       op0=OP.mult, op1=OP.add)

                # DVE: correction crosses cr1 = s01 x m1, cr2 = s23 x m2,
                # cc = cr1 + 0.5 cr2, b = V + dt * cc
                cr1h = hplane(6, "cr1h")[:, :, :, 0:G]
                cr2h = hplane(7, "cr2h")[:, :, :, 0:G]
                cross_into(V, [cr1h[:, :, ci, :] for ci in range(3)],
                           [s01h[:, :, ci, :] for ci in range(3)],
                           [m1h[:, :, ci, :] for ci in range(3)], V_tmp)
                cross_into(V, [cr2h[:, :, ci, :] for ci in range(3)],
                           [s23h[:, :, ci, :] for ci in range(3)],
                           [m2h[:, :, ci, :] for ci in range(3)], V_tmp)
                V.tensor_tensor(out=fuse(cr1h), in0=fuse(cr2h),
                                in1=fuse(cr1h), op=OP.add)
                V.scalar_tensor_tensor(out=fuse(vv), in0=fuse(cr1h),
                                       scalar=DT / 2.0, in1=fuse(vv),
                                       op0=OP.mult, op1=OP.add)

                # ---- SA1: n=4 -> 8 (bf16 outputs for theta/b) ----
                th8b = half_pool.tile(pshape, BF16, tag="h8", name="th8")
                th8 = th8b[:, :, :, 0:U8]
                b8 = th8b[:, :, :, 64:64 + U8]
                r8 = plane(7, "r8")[:, :, :, 0:U8]
                csa = hplane(5, "csa")[:, :, :, 0:U8]     # c3h dead
                csb = hplane(6, "csb")[:, :, :, 0:U8]     # cr1h dead
                sa_level(th4, vv, rho4, 4, th8, b8, r8, csa, csb, btmp(U8))

                # ---- SA2: n=8 -> 16, into f32 accumulators ----
                oth = th16[:, :, :, u16o:u16o + U16]
                ob = b16[:, :, :, u16o:u16o + U16]
                orr = r16[:, :, :, u16o:u16o + U16]
                cs2t = hplane(7, "cs2t")                  # cr2h dead
                csa2 = cs2t[:, :, :, 0:U16]
                csb2 = cs2t[:, :, :, 64:64 + U16]
                sa_level(th8, b8, r8, 8, oth, ob, orr, csa2, csb2, btmp(U16))
                s0 += S_c

            # ---- batched SA levels: 16 -> 32 -> 64 -> 128 -> 256 ----
            cur = (th16[:], b16[:], r16[:])
            nu = NU16
            bufs = [(1, 2, 3), (4, 5, 6), (1, 2, 3)]
            n_steps = 16
            for lev in range(4):
                nu //= 2
                if lev < 3:
                    bq = bufs[lev]
                    nth = plane(bq[0], f"th{n_steps*2}")[:, :, :, 0:nu]
                    nb = plane(bq[1], f"b{n_steps*2}")[:, :, :, 0:nu]
                    nr = plane(bq[2], f"r{n_steps*2}")[:, :, :, 0:nu]
                else:
                    nth = plane(4, "th256")[:, :, :, 0:nu]
                    bd256 = tree_pool.tile([128, 2, g, 3, nu], F32,
                                           tag="bd256", name="bd256")
                    nb = bd256[:, 0]
                    nr = bd256[:, 1]
                csa = hplane(5, f"ua{lev}")[:, :, :, 0:nu]
                csb = hplane(6, f"ub{lev}")[:, :, :, 0:nu]
                sa_level(cur[0], cur[1], cur[2], n_steps, nth, nb, nr,
                         csa, csb, btmp(nu))
                cur = (nth, nb, nr)
                n_steps *= 2

            # ---- convert 256-step units to quaternions ----
            NU = 8
            th256 = cur[0]
            K2 = (DT / 2.0) ** 2
            h2 = tree_pool.tile([128, g, NU], F32, tag="h2", name="h2")[:]
            hy = tree_pool.tile([128, g, NU], F32, tag="hy", name="hy")[:]
            hz = tree_pool.tile([128, g, NU], F32, tag="hz", name="hz")[:]
            q256 = tree_pool.tile([128, g, 4, NU], F32, tag="q256",
                                  name="q256")
            thc = [th256[:, :, ci, :] for ci in range(3)]
            V.tensor_tensor(out=h2, in0=thc[0], in1=thc[0], op=OP.mult)
            for ci in (1, 2):
                V.tensor_tensor(out=hy, in0=thc[ci], in1=thc[ci], op=OP.mult)
                V.tensor_tensor(out=h2, in0=h2, in1=hy, op=OP.add)
            # qw = 1 - (k/2) h2 + (k^2/24) h2^2
            V.scalar_tensor_tensor(out=hy, in0=h2, scalar=K2 * K2 / 24.0,
                                   in1=h2, op0=OP.mult, op1=OP.mult)
            V.scalar_tensor_tensor(out=hz, in0=h2, scalar=-K2 / 2.0,
                                   in1=hy, op0=OP.mult, op1=OP.add)
            Sc.activation(q256[:, :, 0, :], hz, AF.Identity, bias=1.0)
            # qv = (dt/2)(1 - (k/6) h2 + (k^2/120) h2^2) * theta
            V.scalar_tensor_tensor(out=hy, in0=h2, scalar=K2 * K2 / 120.0,
                                   in1=h2, op0=OP.mult, op1=OP.mult)
            V.scalar_tensor_tensor(out=hz, in0=h2, scalar=-K2 / 6.0,
                                   in1=hy, op0=OP.mult, op1=OP.add)
            Sc.activation(hz, hz, AF.Copy, scale=DT / 2.0, bias=DT / 2.0)
            for ci in range(3):
                V.tensor_tensor(out=q256[:, :, 1 + ci, :],
                                in0=thc[ci], in1=hz, op=OP.mult)

            # ---- quaternion binary tree over 8 units (3 levels) ----
            qt = q256
            bdt = bd256
            n2 = 256.0
            n_lvl = int(math.log2(NU))
            for lvl in range(1, n_lvl + 1):
                n = NU >> lvl
                nqt = tree_pool.tile([128, g, 4, n], F32, tag=f"nq{lvl}",
                                     name=f"nq{lvl}")
                nbd = tree_pool.tile([128, 2, g, 3, n], F32, tag=f"nbd{lvl}",
                                     name=f"nbd{lvl}")
                P = [plane_pool.tile([128, g, 4, n], F32, tag=f"q{a + 1}",
                                     name=f"P{a}")[:] for a in range(4)]
                tcc = plane_pool.tile([128, 2, g, 3, n], F32, tag="q5",
                                      name="tcc")
                tcq = plane_pool.tile([128, 2, g, 3, n], F32, tag="q6",
                                      name="tcq")
                tcw = plane_pool.tile([128, 2, g, 3, n], F32, tag="q7",
                                      name="tcw")
                tms = [xtmp_pool.tile([128, g, GMAX], F32, tag="vt",
                                      name="tms")[:, :, 0:2 * n].rearrange(
                    "p g (s n) -> p g s n", s=2).transpose([0, 2, 1, 3])
                    for _ in range(2)]

                q2all = qt[:, :, :, 1::2]
                for a in range(4):
                    q1a = qt[:, :, a, 0::2].unsqueeze(2).broadcast_to(
                        [128, g, 4, n])
                    V.tensor_tensor(out=P[a], in0=q1a, in1=q2all, op=OP.mult)
                for c in range(4):
                    idx = QIDX[c]
                    sgn = QSGN[c]
                    acc = nqt[:, :, c, :]
                    V.tensor_tensor(
                        out=acc, in0=P[0][:, :, idx[0], :],
                        in1=P[1][:, :, idx[1], :],
                        op=OP.add if sgn[1] > 0 else OP.subtract)
                    for t in (2, 3):
                        V.tensor_tensor(
                            out=acc, in0=acc, in1=P[t][:, :, idx[t], :],
                            op=OP.add if sgn[t] > 0 else OP.subtract)

                def qb(comp):
                    return qt[:, :, comp, 0::2].unsqueeze(1).broadcast_to(
                        [128, 2, g, n])

                def bd2(comp):
                    return bdt[:, :, :, comp, 1::2]

                # tcc = qv1 x bd2 ; tcq = q0 * bd2 ; inner = tcc + tcq
                for i in range(3):
                    i1, i2 = (i + 1) % 3, (i + 2) % 3
                    V.tensor_tensor(out=tms[0], in0=qb(1 + i1), in1=bd2(i2),
                                    op=OP.mult)
                    V.tensor_tensor(out=tms[1], in0=qb(1 + i2), in1=bd2(i1),
                                    op=OP.mult)
                    V.tensor_tensor(out=tcc[:, :, :, i, :], in0=tms[0],
                                    in1=tms[1], op=OP.subtract)
                for i in range(3):
                    V.tensor_tensor(out=tcq[:, :, :, i, :], in0=qb(0),
                                    in1=bd2(i), op=OP.mult)
                V.tensor_tensor(out=tcc[:].opt(), in0=tcc[:].opt(),
                                in1=tcq[:].opt(), op=OP.add)
                # tcw = qv1 x inner
                for i in range(3):
                    i1, i2 = (i + 1) % 3, (i + 2) % 3
                    V.tensor_tensor(out=tms[0], in0=qb(1 + i1),
                                    in1=tcc[:, :, :, i2, :], op=OP.mult)
                    V.tensor_tensor(out=tms[1], in0=qb(1 + i2),
                                    in1=tcc[:, :, :, i1, :], op=OP.mult)
                    V.tensor_tensor(out=tcw[:, :, :, i, :], in0=tms[0],
                                    in1=tms[1], op=OP.subtract)
                # nbd = bd1 + bd2 + 2*tcw  (+ n2*b1 on the d slot)
                V.tensor_tensor(out=nbd[:].opt(),
                                in0=bdt[:, :, :, :, 0::2].opt(),
                                in1=bdt[:, :, :, :, 1::2].opt(), op=OP.add)
                V.scalar_tensor_tensor(out=nbd[:].opt(), in0=tcw[:].opt(),
                                       scalar=2.0, in1=nbd[:].opt(),
                                       op0=OP.mult, op1=OP.add)
                V.scalar_tensor_tensor(
                    out=nbd[:, 1].rearrange("p g c n -> p (g c) n"),
                    in0=bdt[:, 0, :, :, 0::2].rearrange(
                        "p g c n -> p (g c) n"),
                    scalar=n2,
                    in1=nbd[:, 1].rearrange("p g c n -> p (g c) n"),
                    op0=OP.mult, op1=OP.add)

                qt, bdt = nqt, nbd
                n2 *= 2.0

            cur_q = [qt[:, :, ci, :] for ci in range(4)]
            cur_b = [bdt[:, 0, :, ci, :] for ci in range(3)]
            cur_d = [bdt[:, 1, :, ci, :] for ci in range(3)]

            # ---- finalize ----
            out_t = tree_pool.tile([128, g, 7], F32, tag="outt",
                                   name="outt")[:]
            tf = tree_pool.tile([128, g, 1], F32, tag="tf", name="tf")[:]
            for i in range(3):
                V.scalar_tensor_tensor(out=tf, in0=cur_b[i], scalar=1.5,
                                       in1=cur_d[i], op0=OP.mult, op1=OP.add)
                V.tensor_scalar(out=out_t[:, :, i:i + 1], in0=tf,
                                scalar1=DT * DT, scalar2=None, op0=OP.mult)
            sg = tree_pool.tile([128, g, 1], F32, tag="sg", name="sg")[:]
            V.tensor_scalar(out=sg, in0=cur_q[0], scalar1=0.0, scalar2=2.0,
                            op0=OP.is_ge, op1=OP.mult)
            V.tensor_scalar(out=sg, in0=sg, scalar1=-1.0, scalar2=None,
                            op0=OP.add)
            for i in range(4):
                V.tensor_tensor(out=out_t[:, :, 3 + i:4 + i], in0=cur_q[i],
                                in1=sg, op=OP.mult)

            ov = out.rearrange("(g p) c -> g p c", g=g)
            for gi in range(g):
                nc.sync.dma_start(out=ov[gi], in_=out_t[:, gi, :])

    nc.compile()
    return nc


_NC_CACHE = {}
LAST_RESULTS = None


def _ensure_profiling_hooks():
    """Best-effort: provide the antenv.axon_hooks shim + skip S3 upload so
    trace=True works in this stripped container. No-op on failure."""
    import sys
    import types
    try:
        if "antenv.axon_hooks" not in sys.modules:
            from trn_agent_boot.trn_boot import _ntff_profile_via_ctypes
            hook = _ntff_profile_via_ctypes("/opt/axon/libaxon_pjrt.so")
            mod = types.ModuleType("antenv.axon_hooks")
            mod._hook = hook
            mod.get_axon_ntff_profile_hook = lambda: mod._hook
            mod.set_axon_ntff_profile_hook = lambda h: setattr(mod, "_hook", h)
            sys.modules["antenv.axon_hooks"] = mod
        import concourse.bass_utils as bu
        bu.upload_artifacts = lambda tmpdir: tmpdir
    except Exception as e:  # pragma: no cover
        print(f"profiling hook setup failed ({e}); tracing may be skipped")


def kernel(input_seq: np.ndarray) -> np.ndarray:
    from concourse.bass_utils import run_bass_kernel_spmd

    global LAST_RESULTS
    input_seq = np.ascontiguousarray(np.asarray(input_seq, dtype=np.float32))
    assert input_seq.shape == (B_FULL, S_FULL, C), input_seq.shape

    if "nc" not in _NC_CACHE:
        _NC_CACHE["nc"] = build_nc()
    nc = _NC_CACHE["nc"]

    in_maps = [{"x": input_seq[i * R:(i + 1) * R]} for i in range(N_CORES)]
    trace = os.environ.get("BASS_KERNEL_TRACE", "0") == "1"
    if trace:
        _ensure_profiling_hooks()
    try:
        res = run_bass_kernel_spmd(nc, in_maps, core_ids=list(range(N_CORES)),
                                   trace=trace)
    except Exception:
        # transient device wedge (NRT_EXEC_UNIT_UNRECOVERABLE) recovers on
        # a clean re-run; retry once
        res = run_bass_kernel_spmd(nc, in_maps, core_ids=list(range(N_CORES)),
                                   trace=trace)
    LAST_RESULTS = res
    return np.concatenate([r["out"] for r in res.results], axis=0)


# revision 21
# speedup vs baseline: 1.4024x; 1.3328x over previous
"""Trainium2 Bass kernel for the IMU preintegration module.

Full inputs in, full outputs out; internally data-parallel over 8 NeuronCores
(512 batch rows per core).

Math: the scan's per-row state composes associatively as (R, b, d) with
    b = sum_t R_{1..t} a_t,   d = sum_t (S-1-t) R_{1..t} a_t   (raw units;
physical factors of dt are folded into scalars).
  L0: groups of n0=4 steps reduce to (theta, b, rho) with first-order
      rotation.  theta = s01+s23 (+ (dt/2) s01 x s23 BCH term) where
      s01/s23 are the step-pair sums of w.  The b correction
      (dt/4)(theta x u + p x aw) == (dt/2)[s01 x m1 + s23 x m2] with
      m1 = 2V - A0, m2 = A2 + 2 A3 (V = 4-step a-sum, Ai per-phase a).
      rho = 3 A0 + 2 A1 + A2.
  SA: small-angle composition up to 256-step units:
      theta' = tA+tB + (dt/2) tA x tB,  b' = bA+bB + dt (tA x bB),
      rho' = rA + n bA + rB.
  QT: convert theta -> quaternion (2-3 term polys in h = |theta dt/2|^2),
      exact quaternion binary tree for the remaining 3 levels.

Perf design (hardware-measured):
 - The host pre-permutes each slab to [4 phase][6 ch][G] blocks so every
   on-chip read is dense (strided SBUF reads waste read-port bandwidth
   and inflate concurrently-running ops on other engines 2-6x).
 - ScalarE (contention-immune) casts each slab to bf16 once; the whole
   L0 runs in bf16 2x mode on DVE.
 - The three L0 crosses are packed into one 3G-wide cross (9 ops instead
   of 27); SA-level crosses pack their operands via ScalarE copies.
 - GpSimd is idle during the slab phase: measured, its fat TT passes
   inflate concurrent DVE ops by more than the work they remove.
"""

import math
import os
import numpy as np

import concourse.mybir as mybir
from concourse import bass, bacc
from concourse.tile import TileContext

F32 = mybir.dt.float32
BF16 = mybir.dt.bfloat16
OP = mybir.AluOpType
AF = mybir.ActivationFunctionType

# problem constants (hardcoded per harness contract)
B_FULL = 4096
S_FULL = 2048
C = 6
N_CORES = 8
R = B_FULL // N_CORES          # rows per core = 512
DT = float(np.float32(1.0 / 200.0))

QSGN = [(1, -1, -1, -1), (1, 1, 1, -1), (1, -1, 1, 1), (1, 1, -1, 1)]
QIDX = [(0, 1, 2, 3), (1, 0, 3, 2), (2, 3, 0, 1), (3, 2, 1, 0)]

# step chunks per slab: short first chunk so compute starts early, short
# last chunk so the tree phase starts early
CHUNKS = [128, 384, 512, 512, 512]
assert sum(CHUNKS) == S_FULL
NCOL = S_FULL * C


def host_permute(x):
    """[B, S, 6] -> [B, S*6] with per-slab [phase(4), ch(6), G] blocks."""
    B = x.shape[0]
    outp = np.empty((B, NCOL), np.float32)
    s0 = 0
    for S_c in CHUNKS:
        G = S_c // 4
        blk = x[:, s0:s0 + S_c, :].reshape(B, G, 4, C).transpose(0, 2, 3, 1)
        outp[:, s0 * C:(s0 + S_c) * C] = blk.reshape(B, S_c * C)
        s0 += S_c
    return outp


def fuse(ap):
    # [128, g, c, n] -> [128, g*c, n] so stt sees <= 2 free dims
    return ap.rearrange("p g c n -> p (g c) n")


def build_nc(rows=R, s_len=S_FULL):
    nc = bacc.Bacc(None, target_bir_lowering=False, debug=False)
    g = rows // 128                    # 4 rows per partition
    GMAX = 128
    NU16 = s_len // 16                 # 128 16-step units per row
    x = nc.dram_tensor("x", [rows, NCOL], F32, kind="ExternalInput")
    out = nc.dram_tensor("out", [rows, 7], F32, kind="ExternalOutput")
    xv = x.rearrange("(g p) n -> g p n", g=g)

    V = nc.vector
    Gp = nc.gpsimd
    Sc = nc.scalar

    def cross_into(dst, u, v, tmps):
        """dst = u x v per-component (lists of comp views). 9 ops."""
        for i in range(3):
            i1, i2 = (i + 1) % 3, (i + 2) % 3
            V.tensor_tensor(out=tmps[0], in0=u[i1], in1=v[i2], op=OP.mult)
            V.tensor_tensor(out=tmps[1], in0=u[i2], in1=v[i1], op=OP.mult)
            V.tensor_tensor(out=dst[i], in0=tmps[0], in1=tmps[1],
                            op=OP.subtract)

    with TileContext(nc) as tc:
        with (
            tc.tile_pool(name="slab", bufs=1) as slab_pool,
            tc.tile_pool(name="slabh", bufs=2) as slabh_pool,
            tc.tile_pool(name="plane", bufs=1) as plane_pool,
            tc.tile_pool(name="pack", bufs=1) as pack_pool,
            tc.tile_pool(name="upper", bufs=1) as upper_pool,
            tc.tile_pool(name="xtmp", bufs=2) as xtmp_pool,
            tc.tile_pool(name="acc", bufs=1) as acc_pool,
            tc.tile_pool(name="tree", bufs=1) as tree_pool,
        ):
            pshape = [128, g, 3, GMAX]

            def hplane(tg, nm, n=GMAX):
                return plane_pool.tile([128, g, 3, n], BF16, tag=tg,
                                       name=nm)

            def btmp(n):
                return [xtmp_pool.tile([128, g, 3 * GMAX], BF16, tag="vt",
                                       name="vt")[:, :, 0:n]
                        for _ in range(2)]

            # persistent accumulators: 16-step units across all slabs
            th16 = acc_pool.tile(pshape, F32, tag="t16", name="t16")
            b16 = acc_pool.tile(pshape, F32, tag="bb16", name="bb16")
            r16 = acc_pool.tile(pshape, F32, tag="r16", name="r16")

            def sa_level(thI, bI, rI, n_steps, oth, ob, orr, cst, tmps,
                         pk=None, radd_eng=V):
                """Compose adjacent units (width-2U 4D views -> width U).
                cst: packed [128,g,3,2U] tile for [csa|csb].  If pk is
                given (PL, PR packed [128,g,3,2U] bf16 tiles), ScalarE
                packs [thA|thA] and [thB|bB] dense first and the cross
                runs as one wide op; else two strided crosses."""
                U = oth.shape[3]
                csa, csb = cst[:, :, :, 0:U], cst[:, :, :, U:2 * U]
                thA = [thI[:, :, ci, 0::2] for ci in range(3)]
                if pk is not None:
                    PL, PR = pk
                    for ci in range(3):
                        Sc.copy(PL[:, :, ci, 0:U], thI[:, :, ci, 0::2])
                        Sc.copy(PL[:, :, ci, U:2 * U], thI[:, :, ci, 0::2])
                        Sc.copy(PR[:, :, ci, 0:U], thI[:, :, ci, 1::2])
                        Sc.copy(PR[:, :, ci, U:2 * U], bI[:, :, ci, 1::2])
                    cross_into([cst[:, :, ci, :] for ci in range(3)],
                               [PL[:, :, ci, :] for ci in range(3)],
                               [PR[:, :, ci, :] for ci in range(3)],
                               btmp(2 * U))
                else:
                    thB = [thI[:, :, ci, 1::2] for ci in range(3)]
                    bB = [bI[:, :, ci, 1::2] for ci in range(3)]
                    cross_into([csa[:, :, ci, :] for ci in range(3)],
                               thA, thB, tmps)
                    cross_into([csb[:, :, ci, :] for ci in range(3)],
                               thA, bB, tmps)
                V.tensor_tensor(out=fuse(oth), in0=fuse(thI)[:, :, 0::2],
                                in1=fuse(thI)[:, :, 1::2], op=OP.add)
                V.scalar_tensor_tensor(out=fuse(oth), in0=fuse(csa),
                                       scalar=DT / 2.0, in1=fuse(oth),
                                       op0=OP.mult, op1=OP.add)
                V.tensor_tensor(out=fuse(ob), in0=fuse(bI)[:, :, 0::2],
                                in1=fuse(bI)[:, :, 1::2], op=OP.add)
                V.scalar_tensor_tensor(out=fuse(ob), in0=fuse(csb),
                                       scalar=DT, in1=fuse(ob),
                                       op0=OP.mult, op1=OP.add)
                radd_eng.tensor_tensor(out=fuse(orr),
                                       in0=fuse(rI)[:, :, 0::2],
                                       in1=fuse(rI)[:, :, 1::2], op=OP.add)
                V.scalar_tensor_tensor(out=fuse(orr),
                                       in0=fuse(bI)[:, :, 0::2],
                                       scalar=float(n_steps), in1=fuse(orr),
                                       op0=OP.mult, op1=OP.add)

            s0 = 0
            for si, S_c in enumerate(CHUNKS):
                G = S_c // 4
                U8 = S_c // 8
                U16 = S_c // 16
                u16o = s0 // 16

                sf = slab_pool.tile([128, g, 512 * C], F32, tag="slab",
                                    name=f"slab{si}")
                sh = slabh_pool.tile([128, g, 512 * C], BF16, tag="slabh",
                                     name=f"slabh{si}")
                for gi in range(g):
                    nc.sync.dma_start(
                        out=sf[:, gi, 0:S_c * C],
                        in_=xv[gi, :, s0 * C:(s0 + S_c) * C])
                # ScalarE: cast each phase block to bf16 (dense)
                for ph in range(4):
                    Sc.copy(sh[:, :, ph * 6 * G:(ph + 1) * 6 * G],
                            sf[:, :, ph * 6 * G:(ph + 1) * 6 * G])
                shv = sh[:, :, 0:S_c * C].rearrange(
                    "p g (ph ch n) -> p g ph ch n", ph=4, ch=C)
                W = [shv[:, :, i, 0:3, :] for i in range(4)]
                At = [shv[:, :, i, 3:6, :] for i in range(4)]

                # packed cross operands: Lh = [s01|s01|s23],
                # Rh = [s23|m1|m2], Ch = [c3|cr1|cr2]
                Lh = pack_pool.tile([128, g, 3, 3 * GMAX], BF16, tag="Lh",
                                    name="Lh")
                Rh = pack_pool.tile([128, g, 3, 3 * GMAX], BF16, tag="Rh",
                                    name="Rh")
                Ch = pack_pool.tile([128, g, 3, 3 * GMAX], BF16, tag="Ch",
                                    name="Ch")
                s01 = Lh[:, :, :, 0:G]
                s23 = Lh[:, :, :, 2 * G:3 * G]
                th4 = hplane("th4", "th4")[:, :, :, 0:G]
                V.tensor_tensor(out=s01, in0=W[0], in1=W[1], op=OP.add)
                V.tensor_tensor(out=s23, in0=W[2], in1=W[3], op=OP.add)
                for ci in range(3):
                    Sc.copy(Lh[:, :, ci, G:2 * G], s01[:, :, ci, :])
                    Sc.copy(Rh[:, :, ci, 0:G], s23[:, :, ci, :])
                V.tensor_tensor(out=fuse(th4), in0=fuse(s01), in1=fuse(s23),
                                op=OP.add)

                # V-chain + m1 = 2V - A0, m2 = A2 + 2 A3 (all bf16 2x).
                # At views can't fuse (g,c) across the phase dim, so TT
                # runs 4D (3 free dims) and stt goes per-component.
                vv = hplane("vv", "vv")[:, :, :, 0:G]
                V.tensor_tensor(out=vv, in0=At[0], in1=At[1], op=OP.add)
                V.tensor_tensor(out=vv, in0=vv, in1=At[2], op=OP.add)
                V.tensor_tensor(out=vv, in0=vv, in1=At[3], op=OP.add)
                m1h = Rh[:, :, :, G:2 * G]
                m2h = Rh[:, :, :, 2 * G:3 * G]
                r1 = hplane("rr", "r1")[:, :, :, 0:G]
                rho4 = hplane("rho4", "rho4")[:, :, :, 0:G]
                for ci in range(3):
                    V.scalar_tensor_tensor(out=m1h[:, :, ci, :],
                                           in0=vv[:, :, ci, :], scalar=2.0,
                                           in1=At[0][:, :, ci, :],
                                           op0=OP.mult, op1=OP.subtract)
                    V.scalar_tensor_tensor(out=m2h[:, :, ci, :],
                                           in0=At[3][:, :, ci, :],
                                           scalar=2.0,
                                           in1=At[2][:, :, ci, :],
                                           op0=OP.mult, op1=OP.add)
                    # rho = 3 A0 + 2 A1 + A2
                    V.scalar_tensor_tensor(out=r1[:, :, ci, :],
                                           in0=At[0][:, :, ci, :],
                                           scalar=3.0,
                                           in1=At[2][:, :, ci, :],
                                           op0=OP.mult, op1=OP.add)
                    V.scalar_tensor_tensor(out=rho4[:, :, ci, :],
                                           in0=At[1][:, :, ci, :],
                                           scalar=2.0,
                                           in1=r1[:, :, ci, :],
                                           op0=OP.mult, op1=OP.add)

                # packed crosses: [c3|cr1|cr2] = [s01|s01|s23] x
                # [s23|m1|m2] as one 3G-wide cross (9 ops, not 27)
                cross_into([Ch[:, :, ci, 0:3 * G] for ci in range(3)],
                           [Lh[:, :, ci, 0:3 * G] for ci in range(3)],
                           [Rh[:, :, ci, 0:3 * G] for ci in range(3)],
                           btmp(3 * G))
                c3h = Ch[:, :, :, 0:G]
                cr1 = Ch[:, :, :, G:2 * G]
                cr2 = Ch[:, :, :, 2 * G:3 * G]
                V.scalar_tensor_tensor(out=fuse(th4), in0=fuse(c3h),
                                       scalar=DT / 2.0, in1=fuse(th4),
                                       op0=OP.mult, op1=OP.add)
                V.tensor_tensor(out=fuse(cr1), in0=fuse(cr2),
                                in1=fuse(cr1), op=OP.add)
                V.scalar_tensor_tensor(out=fuse(vv), in0=fuse(cr1),
                                       scalar=DT / 2.0, in1=fuse(vv),
                                       op0=OP.mult, op1=OP.add)

                # ---- SA1: n=4 -> 8 (packed crosses via ScalarE) ----
                th8b = hplane("h8", "th8b")
                th8 = th8b[:, :, :, 0:U8]
                b8 = th8b[:, :, :, 64:64 + U8]
                r8 = hplane("rr", "r8")[:, :, :, 0:U8]
                cs1 = hplane("cs", "cs1")
                pl1 = hplane("pl", "pl1")
                pr1 = hplane("pr", "pr1")
                sa_level(th4, vv, rho4, 4, th8, b8, r8,
                         cs1[:, :, :, 0:2 * U8], None,
                         pk=(pl1[:, :, :, 0:2 * U8], pr1[:, :, :, 0:2 * U8]))

                # ---- SA2: n=8 -> 16, into f32 accumulators ----
                oth = th16[:, :, :, u16o:u16o + U16]
                ob = b16[:, :, :, u16o:u16o + U16]
                orr = r16[:, :, :, u16o:u16o + U16]
                cs2 = hplane("cs", "cs2")
                pl2 = hplane("pl", "pl2")
                pr2 = hplane("pr", "pr2")
                sa_level(th8, b8, r8, 8, oth, ob, orr,
                         cs2[:, :, :, 0:2 * U16], None,
                         pk=(pl2[:, :, :, 0:2 * U16],
                             pr2[:, :, :, 0:2 * U16]))
                s0 += S_c

            # ---- batched SA levels: 16 -> 32 -> 64 -> 128 -> 256 ----
            cur = (th16[:], b16[:], r16[:])
            nu = NU16
            n_steps = 16
            for lev in range(4):
                nu //= 2
                if lev < 3:
                    a = lev % 2
                    nth = upper_pool.tile([128, g, 3, 64], F32,
                                          tag=f"u{a}t", name=f"u{lev}t")[
                        :, :, :, 0:nu]
                    nb = upper_pool.tile([128, g, 3, 64], F32,
                                         tag=f"u{a}b", name=f"u{lev}b")[
                        :, :, :, 0:nu]
                    nr = upper_pool.tile([128, g, 3, 64], F32,
                                         tag=f"u{a}r", name=f"u{lev}r")[
                        :, :, :, 0:nu]
                else:
                    nth = upper_pool.tile([128, g, 3, 8], F32,
                                          tag="th256", name="th256")[:]
                    bd256 = tree_pool.tile([128, 2, g, 3, nu], F32,
                                           tag="bd256", name="bd256")
                    nb = bd256[:, 0]
                    nr = bd256[:, 1]
                cst = hplane("cs", f"ucs{lev}")
                sa_level(cur[0], cur[1], cur[2], n_steps, nth, nb, nr,
                         cst[:, :, :, 0:2 * nu], btmp(nu), radd_eng=Gp)
                cur = (nth, nb, nr)
                n_steps *= 2

            # ---- convert 256-step units to quaternions ----
            NU = 8
            th256 = cur[0]
            K2 = (DT / 2.0) ** 2
            h2 = tree_pool.tile([128, g, NU], F32, tag="h2", name="h2")[:]
            hy = tree_pool.tile([128, g, NU], F32, tag="hy", name="hy")[:]
            hz = tree_pool.tile([128, g, NU], F32, tag="hz", name="hz")[:]
            q256 = tree_pool.tile([128, g, 4, NU], F32, tag="q256",
                                  name="q256")
            thc = [th256[:, :, ci, :] for ci in range(3)]
            V.tensor_tensor(out=h2, in0=thc[0], in1=thc[0], op=OP.mult)
            for ci in (1, 2):
                V.tensor_tensor(out=hy, in0=thc[ci], in1=thc[ci], op=OP.mult)
                V.tensor_tensor(out=h2, in0=h2, in1=hy, op=OP.add)
            V.scalar_tensor_tensor(out=hy, in0=h2, scalar=K2 * K2 / 24.0,
                                   in1=h2, op0=OP.mult, op1=OP.mult)
            V.scalar_tensor_tensor(out=hz, in0=h2, scalar=-K2 / 2.0,
                                   in1=hy, op0=OP.mult, op1=OP.add)
            Sc.activation(q256[:, :, 0, :], hz, AF.Identity, bias=1.0)
            V.scalar_tensor_tensor(out=hy, in0=h2, scalar=K2 * K2 / 120.0,
                                   in1=h2, op0=OP.mult, op1=OP.mult)
            V.scalar_tensor_tensor(out=hz, in0=h2, scalar=-K2 / 6.0,
                                   in1=hy, op0=OP.mult, op1=OP.add)
            Sc.activation(hz, hz, AF.Copy, scale=DT / 2.0, bias=DT / 2.0)
            for ci in range(3):
                V.tensor_tensor(out=q256[:, :, 1 + ci, :],
                                in0=thc[ci], in1=hz, op=OP.mult)

            # ---- quaternion binary tree over 8 units (3 levels) ----
            qt = q256
            bdt = bd256
            n2 = 256.0
            n_lvl = int(math.log2(NU))
            for lvl in range(1, n_lvl + 1):
                n = NU >> lvl
                nqt = tree_pool.tile([128, g, 4, n], F32, tag=f"nq{lvl}",
                                     name=f"nq{lvl}")
                nbd = tree_pool.tile([128, 2, g, 3, n], F32, tag=f"nbd{lvl}",
                                     name=f"nbd{lvl}")
                P = [tree_pool.tile([128, g, 4, n], F32, tag=f"P{a}{lvl}",
                                    name=f"P{a}")[:] for a in range(4)]
                tcc = tree_pool.tile([128, 2, g, 3, n], F32,
                                     tag=f"tcc{lvl}", name="tcc")
                tcq = tree_pool.tile([128, 2, g, 3, n], F32,
                                     tag=f"tcq{lvl}", name="tcq")
                tcw = tree_pool.tile([128, 2, g, 3, n], F32,
                                     tag=f"tcw{lvl}", name="tcw")
                tms = [tree_pool.tile([128, 2, g, n], F32,
                                      tag=f"tm{i}{lvl}", name="tms")[:]
                       for i in range(2)]

                q2all = qt[:, :, :, 1::2]
                for a in range(4):
                    q1a = qt[:, :, a, 0::2].unsqueeze(2).broadcast_to(
                        [128, g, 4, n])
                    V.tensor_tensor(out=P[a], in0=q1a, in1=q2all, op=OP.mult)
                for c in range(4):
                    idx = QIDX[c]
                    sgn = QSGN[c]
                    acc = nqt[:, :, c, :]
                    V.tensor_tensor(
                        out=acc, in0=P[0][:, :, idx[0], :],
                        in1=P[1][:, :, idx[1], :],
                        op=OP.add if sgn[1] > 0 else OP.subtract)
                    for t in (2, 3):
                        V.tensor_tensor(
                            out=acc, in0=acc, in1=P[t][:, :, idx[t], :],
                            op=OP.add if sgn[t] > 0 else OP.subtract)

                def qb(comp):
                    return qt[:, :, comp, 0::2].unsqueeze(1).broadcast_to(
                        [128, 2, g, n])

                def bd2(comp):
                    return bdt[:, :, :, comp, 1::2]

                # bd-rotation chain on GpSimd: independent of the qmul
                # chain (which feeds the next level), so the two engines
                # run concurrently through the tree levels.
                for i in range(3):
                    i1, i2 = (i + 1) % 3, (i + 2) % 3
                    Gp.tensor_tensor(out=tms[0], in0=qb(1 + i1),
                                     in1=bd2(i2), op=OP.mult)
                    Gp.tensor_tensor(out=tms[1], in0=qb(1 + i2),
                                     in1=bd2(i1), op=OP.mult)
                    Gp.tensor_tensor(out=tcc[:, :, :, i, :], in0=tms[0],
                                     in1=tms[1], op=OP.subtract)
                for i in range(3):
                    Gp.tensor_tensor(out=tcq[:, :, :, i, :], in0=qb(0),
                                     in1=bd2(i), op=OP.mult)
                Gp.tensor_tensor(out=tcc[:].opt(), in0=tcc[:].opt(),
                                 in1=tcq[:].opt(), op=OP.add)
                for i in range(3):
                    i1, i2 = (i + 1) % 3, (i + 2) % 3
                    Gp.tensor_tensor(out=tms[0], in0=qb(1 + i1),
                                     in1=tcc[:, :, :, i2, :], op=OP.mult)
                    Gp.tensor_tensor(out=tms[1], in0=qb(1 + i2),
                                     in1=tcc[:, :, :, i1, :], op=OP.mult)
                    Gp.tensor_tensor(out=tcw[:, :, :, i, :], in0=tms[0],
                                     in1=tms[1], op=OP.subtract)
                Gp.tensor_tensor(out=nbd[:].opt(),
                                 in0=bdt[:, :, :, :, 0::2].opt(),
                                 in1=bdt[:, :, :, :, 1::2].opt(), op=OP.add)
                V.scalar_tensor_tensor(out=nbd[:].opt(), in0=tcw[:].opt(),
                                       scalar=2.0, in1=nbd[:].opt(),
                                       op0=OP.mult, op1=OP.add)
                V.scalar_tensor_tensor(
                    out=nbd[:, 1].rearrange("p g c n -> p (g c) n"),
                    in0=bdt[:, 0, :, :, 0::2].rearrange(
                        "p g c n -> p (g c) n"),
                    scalar=n2,
                    in1=nbd[:, 1].rearrange("p g c n -> p (g c) n"),
                    op0=OP.mult, op1=OP.add)

                qt, bdt = nqt, nbd
                n2 *= 2.0

            cur_q = [qt[:, :, ci, :] for ci in range(4)]
            cur_b = [bdt[:, 0, :, ci, :] for ci in range(3)]
            cur_d = [bdt[:, 1, :, ci, :] for ci in range(3)]

            # ---- finalize ----
            out_t = tree_pool.tile([128, g, 7], F32, tag="outt",
                                   name="outt")[:]
            tf = tree_pool.tile([128, g, 1], F32, tag="tf", name="tf")[:]
            for i in range(3):
                V.scalar_tensor_tensor(out=tf, in0=cur_b[i], scalar=1.5,
                                       in1=cur_d[i], op0=OP.mult, op1=OP.add)
                V.tensor_scalar(out=out_t[:, :, i:i + 1], in0=tf,
                                scalar1=DT * DT, scalar2=None, op0=OP.mult)
            sg = tree_pool.tile([128, g, 1], F32, tag="sg", name="sg")[:]
            V.tensor_scalar(out=sg, in0=cur_q[0], scalar1=0.0, scalar2=2.0,
                            op0=OP.is_ge, op1=OP.mult)
            V.tensor_scalar(out=sg, in0=sg, scalar1=-1.0, scalar2=None,
                            op0=OP.add)
            for i in range(4):
                V.tensor_tensor(out=out_t[:, :, 3 + i:4 + i], in0=cur_q[i],
                                in1=sg, op=OP.mult)

            ov = out.rearrange("(g p) c -> g p c", g=g)
            for gi in range(g):
                nc.sync.dma_start(out=ov[gi], in_=out_t[:, gi, :])

    nc.compile()
    return nc


_NC_CACHE = {}
LAST_RESULTS = None


def _ensure_profiling_hooks():
    """Best-effort: provide the antenv.axon_hooks shim + skip S3 upload so
    trace=True works in this stripped container. No-op on failure."""
    import sys
    import types
    try:
        if "antenv.axon_hooks" not in sys.modules:
            from trn_agent_boot.trn_boot import _ntff_profile_via_ctypes
            hook = _ntff_profile_via_ctypes("/opt/axon/libaxon_pjrt.so")
            mod = types.ModuleType("antenv.axon_hooks")
            mod._hook = hook
            mod.get_axon_ntff_profile_hook = lambda: mod._hook
            mod.set_axon_ntff_profile_hook = lambda h: setattr(mod, "_hook", h)
            sys.modules["antenv.axon_hooks"] = mod
        import concourse.bass_utils as bu
        bu.upload_artifacts = lambda tmpdir: tmpdir
    except Exception as e:  # pragma: no cover
        print(f"profiling hook setup failed ({e}); tracing may be skipped")


def kernel(input_seq: np.ndarray) -> np.ndarray:
    from concourse.bass_utils import run_bass_kernel_spmd

    global LAST_RESULTS
    input_seq = np.ascontiguousarray(np.asarray(input_seq, dtype=np.float32))
    assert input_seq.shape == (B_FULL, S_FULL, C), input_seq.shape
    xp = host_permute(input_seq)

    if "nc" not in _NC_CACHE:
        _NC_CACHE["nc"] = build_nc()
    nc = _NC_CACHE["nc"]

    in_maps = [{"x": xp[i * R:(i + 1) * R]} for i in range(N_CORES)]
    trace = os.environ.get("BASS_KERNEL_TRACE", "0") == "1"
    if trace:
        _ensure_profiling_hooks()
    try:
        res = run_bass_kernel_spmd(nc, in_maps, core_ids=list(range(N_CORES)),
                                   trace=trace)
    except Exception:
        # transient device wedge (NRT_EXEC_UNIT_UNRECOVERABLE) recovers on
        # a clean re-run; retry once
        res = run_bass_kernel_spmd(nc, in_maps, core_ids=list(range(N_CORES)),
                                   trace=trace)
    LAST_RESULTS = res
    return np.concatenate([r["out"] for r in res.results], axis=0)
